# revision 22
# baseline (speedup 1.0000x reference)
"""BERT self-attention (B=4, S=2048, H=768, 12 heads x d=64) on 8 Trainium2
NeuronCores.

Sharding: core c handles batch b = c//2 and head group hg = c%2 (6 heads).
Each core computes q/k/v projections for its 6 heads from its batch's
hidden_states, then attention per head. No cross-core communication; the host
scatters inputs and gathers/reassembles the output.

Engine balance (the softmax exp on 25.2M elements is the co-bottleneck with
the PE):
  - PE: projections (KC=6 f32r chunks when biases are zero), scores as
    row-tiled K=64 pairs (two heads concurrently in PE row groups 0-63 /
    64-127), ctx with M=128 stationary (64 v-dims | 64 ones columns; the ones
    leave 64 copies of sumexp in psum rows 64-127).
  - ACT: exp for 12 of 16 j-tiles per chunk (exact, with additive mask as
    activation bias when present).
  - DVE: exp for 4 j-tiles via two chained custom DVE ops computing
    (1 + x*scale/16384)^16384 (error ~x^2/32768, well under bf16 noise),
    plus the finalize: reciprocal_approx_fast on the sumexp rows and one
    [64,512] multiply per head.
  - GPSIMD: all psum->sbuf copies (q/k/v projections) and the ones memset.

PE emission is woven: per (pair, i-chunk) period the stream is
[scores jt | ctx(prev) h0 jt | ctx(prev) h1 jt] x 16 with projection matmuls
injected in early periods, so the PE never waits on the elementwise engines.
"""
import os

import numpy as np

if not os.environ.get("KERNEL_TRACE"):
    os.environ.setdefault("BASS_NEVER_TRACE", "1")

import concourse.bass as bass
import concourse.bass_utils as _bass_utils
import concourse.mybir as mybir
import concourse.tile as tile
from concourse import bacc
from concourse.bass import ts
from concourse.bass_utils import run_bass_kernel_spmd

if os.environ.get("KERNEL_LDW_OPT", "0") == "1" and not getattr(
    _bass_utils, "_ant_ldw_patched", False
):
    # enable the walrus LDWEIGHTS double-buffering optimization: without it
    # every full-row-group matmul serializes its 107ns stationary load
    _orig_run_command = _bass_utils.run_command

    def _ldw_run_command(argv, **kwargs):
        argv = [
            "--enable-ldw-opt=true" if a == "--enable-ldw-opt=false" else a
            for a in argv
        ]
        return _orig_run_command(argv, **kwargs)

    _bass_utils.run_command = _ldw_run_command
    _bass_utils._ant_ldw_patched = True

F32R = mybir.dt.float32r
F32 = mybir.dt.float32
BF16 = mybir.dt.bfloat16

HIDDEN = 768
N_HEADS = 12
HEAD_DIM = 64
B = 4
S = 2048
HPC = 6          # heads per core
NI = S // 512    # 4 i-chunks of 512
NJ = S // 128    # 16 j-tiles of 128
SCALE = 1.0 / np.sqrt(HEAD_DIM)
EXPN = 16384.0   # (1 + x*SCALE/EXPN)^EXPN ~= exp(x*SCALE)
EXPC = SCALE / EXPN

DVE_JTS = tuple(
    int(x) for x in os.environ.get("KERNEL_DVE_JTS", "7,12").split(",") if x != ""
)
DVE_JTS_LATE = tuple(
    int(x)
    for x in os.environ.get("KERNEL_DVE_JTS_LATE", "3,7,11,15").split(",")
    if x != ""
)

_cache = {}
last_results = None
_exp_ops = None


def _to_bf16(x):
    import ml_dtypes

    return np.asarray(x, dtype=np.float32).astype(ml_dtypes.bfloat16)


def _register_exp_ops():
    """Register the two chained custom DVE ops for the approximate exp:
      EXP_A: y = (s1 + x*s0)^64      (affine + 6 squarings, 8 v3 stages)
      EXP_B: y = x^256               (8 squarings, 8 v3 stages)
    Composition: (1 + x*EXPC)^16384. Idempotent across imports."""
    global _exp_ops
    if _exp_ops is not None:
        return _exp_ops
    from concourse import dve_ops
    from concourse.dve_spec import Spec, Src0, C0, C1, _has_src1, lower
    from concourse.dve_uop import DveOpSpec

    def build(name, spec):
        existing = [op for op in dve_ops.OPS if op.name == name]
        if existing:
            return existing[0]
        shas = {}
        for ver in ("v3", "v4"):
            try:
                u = lower(spec, ver=ver)
                shas[ver] = DveOpSpec(
                    name=name, opcode=0, uops=u, rd1_en=_has_src1(spec)
                ).sha(ver)
            except Exception:
                pass
        op = dve_ops.DveOp(name, spec, subdim=False, uops_sha=shas)
        dve_ops.OPS.append(op)
        dve_ops.CUSTOM_DVE_SPECS[name] = spec
        dve_ops._SUB_OPCODE_FOR_NAME[name] = (
            dve_ops._CUSTOM_DVE_ROW_BASE + len(dve_ops.OPS) - 1
        )
        return op

    y = C1 + Src0 * C0
    for _ in range(6):
        y = y * y
    spec_a = Spec(
        body=y,
        reference=lambda in0, in1, s0, s1, imm2: (s1 + in0 * s0) ** 64,
    )
    z = Src0
    for _ in range(8):
        z = z * z
    spec_b = Spec(
        body=z,
        reference=lambda in0, in1, s0, s1, imm2: in0 ** 256,
    )
    _exp_ops = (build("ANT_EXP_A", spec_a), build("ANT_EXP_B", spec_b))
    return _exp_ops


def _build(use_mask: bool, has_bias: bool):
    exp_a, exp_b = _register_exp_ops()
    KC = 7 if has_bias else 6  # contraction chunks; row 768 = ones/bias row

    nc = bacc.Bacc("TRN2", target_bir_lowering=False, debug=False, num_devices=8)

    xT_d = nc.dram_tensor("xT", [KC * 128, S], BF16, kind="ExternalInput")
    wq_d = nc.dram_tensor("wq", [KC * 128, HPC * HEAD_DIM], BF16, kind="ExternalInput")
    wk_d = nc.dram_tensor("wk", [KC * 128, HPC * HEAD_DIM], BF16, kind="ExternalInput")
    wv_d = nc.dram_tensor("wv", [KC * 128, HPC * HEAD_DIM], BF16, kind="ExternalInput")
    if use_mask:
        emb_d = nc.dram_tensor("emb", [128, NJ], F32, kind="ExternalInput")
        em2_d = nc.dram_tensor("em2", [128, NJ], F32, kind="ExternalInput")
    out_d = nc.dram_tensor("out", [HPC, HEAD_DIM, S], F32, kind="ExternalOutput")

    with tile.TileContext(nc) as tc:
        with (
            tc.tile_pool(name="const", bufs=1) as cpool,
            tc.tile_pool(name="qk", bufs=1) as qkpool,
            tc.tile_pool(name="vp", bufs=1) as vpool,
            tc.tile_pool(name="op", bufs=3) as opool,
            tc.tile_pool(name="rp", bufs=2) as rpool,
            tc.tile_pool(name="xw", bufs=1) as xwpool,
            tc.tile_pool(name="ex", bufs=34) as expool,
            tc.tile_pool(name="sc", bufs=3) as scpool,
            tc.tile_pool(name="pss", bufs=3, space="PSUM") as pss,
            tc.tile_pool(name="psc", bufs=2, space="PSUM") as psc,
        ):
            if use_mask:
                emb = cpool.tile([128, NJ], F32)
                em2 = cpool.tile([128, NJ], F32)
                nc.sync.dma_start(emb[:], emb_d[:])
                nc.sync.dma_start(em2[:], em2_d[:])

            qT = qkpool.tile([128, HPC // 2, S], BF16)
            kT = qkpool.tile([128, HPC // 2, S], BF16)
            v = vpool.tile([128, NJ, HPC, 128], BF16)
            nc.vector.memset(v[:, :, :, 0:HEAD_DIM], 1.0)

            xT = xwpool.tile([128, KC, S], BF16)
            wq = xwpool.tile([128, KC, HPC * HEAD_DIM], BF16)
            wk = xwpool.tile([128, KC, HPC * HEAD_DIM], BF16)
            wv = xwpool.tile([128, KC, HPC * HEAD_DIM], BF16)
            for c in range(KC):
                nc.sync.dma_start(xT[:, c, :], xT_d[ts(c, 128), :])
                nc.sync.dma_start(wq[:, c, :], wq_d[ts(c, 128), :])
                nc.sync.dma_start(wk[:, c, :], wk_d[ts(c, 128), :])
            for c in range(KC):
                nc.sync.dma_start(wv[:, c, :], wv_d[ts(c, 128), :])

            # warm-up: keep the PE busy during the input DMA window so the
            # HAM clock gate is released before real matmuls start; fp32
            # operands run at 4 cycles/row so 14 matmuls span ~13us; results
            # are discarded (psum slot recycles into the pool)
            wsrc = cpool.tile([128, 640], F32)
            nc.vector.memset(wsrc[:], 0.0)
            wps = pss.tile([128, 2, 512], F32, tag="s", name="warm")
            for wi in range(0):
                nc.tensor.matmul(
                    wps[:, wi % 2, :], wsrc[:, 0:128], wsrc[:, 128:640],
                    start=True, stop=True,
                )

            def emit_qk_mms(p, which):
                # projection matmul thunks, one per (dst, half, c-pair):
                # stationary w chunk [128,128], moving xT [128,512]
                thunks = []
                for w_, dst in [((wq, qT), (wk, kT))[w] for w in which]:
                    for half in range(2):
                        def grp(w_=w_, dst=dst, half=half):
                            acc = pss.tile([128, 2, 512], F32, tag="s")
                            for c in range(KC):
                                for n2 in range(2):
                                    n = 2 * half + n2
                                    nc.tensor.matmul(
                                        acc[:, n2, :], w_[:, c, ts(p, 128)],
                                        xT[:, c, ts(n, 512)],
                                        start=(c == 0), stop=(c == KC - 1),
                                    )
                            nc.vector.tensor_copy(
                                dst[:, p, ts(half, 1024)],
                                acc[:].rearrange("p a n -> p (a n)"),
                            )
                        thunks.append(grp)
                return thunks

            def emit_v_mms(jts):
                thunks = []
                for jt in jts:
                    def grp(jt=jt):
                        pv = pss.tile([128, HPC * HEAD_DIM], F32, tag="s")
                        for c in range(KC):
                            nc.tensor.matmul(
                                pv[:], xT[:, c, ts(jt, 128)], wv[:, c, :],
                                start=(c == 0), stop=(c == KC - 1),
                            )
                        nc.vector.tensor_copy(
                            v[:, jt, :, HEAD_DIM:128],
                            pv[:].rearrange("p (h e) -> p h e", h=HPC),
                        )
                    thunks.append(grp)
                return thunks

            def emit_scores_exp(pr_, ic, jt, dve_set):
                # scores for both heads of the pair as row-tiled K=64 matmuls
                ss = pss.tile([128, 2, 512], F32, tag="s")
                for a_ in range(2):
                    po = 64 * a_
                    nc.tensor.matmul(
                        ss[:, a_, :],
                        kT[po:po + 64, pr_, ts(jt, 128)],
                        qT[po:po + 64, pr_, ts(ic, 512)],
                        start=True, stop=True,
                    )
                ex = expool.tile([128, 2, 512], BF16, tag="e")
                if jt in dve_set:
                    sct = scpool.tile([128, 1024], F32, tag="sc")
                    s1 = em2[:, jt:jt + 1] if use_mask else 1.0
                    nc.vector._custom_dve(
                        exp_a, out=sct[:],
                        in0=ss[:].rearrange("p a n -> p (a n)"),
                        s0=EXPC, s1=s1, imm2=0.0,
                    )
                    nc.vector._custom_dve(
                        exp_b, out=ex[:].rearrange("p a n -> p (a n)"),
                        in0=sct[:], s0=0.0, s1=0.0, imm2=0.0,
                    )
                else:
                    bias = emb[:, jt:jt + 1] if use_mask else 0.0
                    nc.scalar.activation(
                        ex[:], ss[:],
                        mybir.ActivationFunctionType.Exp,
                        scale=SCALE, bias=bias,
                    )
                return ex

            ctx_pc = {}

            def emit_ctx_mm(pr_, a_, jt, ex):
                h = 2 * pr_ + a_
                if jt == 0:
                    ctx_pc[a_] = psc.tile([128, 512], F32, tag="c", name="pc")
                nc.tensor.matmul(
                    ctx_pc[a_][:], v[:, jt, h, :], ex[:, a_, :],
                    start=(jt == 0), stop=(jt == NJ - 1),
                )

            fin_mode = os.environ.get("KERNEL_FINALIZE", "fast")

            def emit_finalize(pr_, ic, heads=(0, 1)):
                for a_ in heads:
                    h = 2 * pr_ + a_
                    pc = ctx_pc[a_]
                    rc = rpool.tile([64, 512], F32, tag="rc")
                    if fin_mode == "fast":
                        nc.vector.reciprocal_approx_fast(rc[:], pc[0:64, :])
                    else:
                        nc.vector.reciprocal(rc[:], pc[0:64, :])
                    o = opool.tile([64, 512], F32, tag="o")
                    nc.vector.tensor_tensor(
                        o[:], pc[64:128, :], rc[:], op=mybir.AluOpType.mult
                    )
                    nc.sync.dma_start(out_d[h, :, ts(ic, 512)], o[:])

            # ---- emission schedule ----
            # pre-loop: full q/k projection for pair 0 (k first: scores need
            # the full kT stationary, q half 0 feeds i-chunks 0-1)
            for t in emit_qk_mms(0, (1, 0)):
                t()

            chunks = [(p_, i_) for p_ in range(HPC // 2) for i_ in range(NI)]
            # projection thunks woven into early periods (v for all j-tiles
            # must land before ctx of chunk 0 runs in period 1)
            fill = {
                0: emit_v_mms(range(0, NJ)),
                1: emit_qk_mms(1, (1,)),
                2: emit_qk_mms(1, (0,)),
                3: emit_qk_mms(2, (1,)),
                4: emit_qk_mms(2, (0,)),
            }
            prev = None
            for ci, ch in enumerate(chunks):
                pr_, ic = ch
                fill_t = list(fill.get(ci, []))
                # weave: scores(chunk ci) jt | ctx(chunk ci-1) h0/h1 jt | fill
                nfill = len(fill_t)
                if ci == 0:
                    dve_set = ()
                elif ci < 5:
                    dve_set = DVE_JTS
                else:
                    dve_set = DVE_JTS_LATE
                exs = []
                for jt in range(NJ):
                    exs.append(emit_scores_exp(pr_, ic, jt, dve_set))
                    if prev is not None:
                        # head 0's ctx runs in the first half of the period and
                        # finalizes mid-period, freeing its psum bank well
                        # before the next period's ctx needs it; head 1 second
                        a_, j0 = (0, 2 * jt) if jt < 8 else (1, 2 * (jt - 8))
                        emit_ctx_mm(prev[0][0], a_, j0, prev[1][j0])
                        emit_ctx_mm(prev[0][0], a_, j0 + 1, prev[1][j0 + 1])
                        if jt == 7:
                            emit_finalize(prev[0][0], prev[0][1], heads=(0,))
                        elif jt == 15:
                            emit_finalize(prev[0][0], prev[0][1], heads=(1,))
                    # inject projection work spread across the period
                    target = nfill * (jt + 1) // NJ
                    while nfill - len(fill_t) < target:
                        fill_t.pop(0)()
                while fill_t:
                    fill_t.pop(0)()
                prev = (ch, exs)
            # drain: ctx + finalize for the last chunk
            for a_ in range(2):
                for jt in range(NJ):
                    emit_ctx_mm(prev[0][0], a_, jt, prev[1][jt])
                emit_finalize(prev[0][0], prev[0][1], heads=(a_,))

    nc.compile()
    return nc


def _get_nc(use_mask: bool, has_bias: bool):
    key = (use_mask, has_bias)
    if key not in _cache:
        _cache[key] = _build(use_mask, has_bias)
    return _cache[key]


def kernel(hidden_states, attention_mask, Wq, bq, Wk, bk, Wv, bv):
    global last_results
    hidden_states = np.asarray(hidden_states, dtype=np.float32)
    attention_mask = np.asarray(attention_mask, dtype=np.float32)
    Wq = np.asarray(Wq, dtype=np.float32)
    Wk = np.asarray(Wk, dtype=np.float32)
    Wv = np.asarray(Wv, dtype=np.float32)
    bq = np.asarray(bq, dtype=np.float32)
    bk = np.asarray(bk, dtype=np.float32)
    bv = np.asarray(bv, dtype=np.float32)

    use_mask = bool(np.any(attention_mask))
    has_bias = bool(np.any(bq) or np.any(bk) or np.any(bv))
    nc = _get_nc(use_mask, has_bias)
    KC = 7 if has_bias else 6

    in_maps = []
    for c in range(8):
        b = c // 2
        hg = c % 2
        cs = slice(hg * HPC * HEAD_DIM, (hg + 1) * HPC * HEAD_DIM)

        xT = np.zeros((KC * 128, S), dtype=np.float32)
        xT[:HIDDEN] = hidden_states[b].T
        if has_bias:
            xT[HIDDEN] = 1.0
        xT = _to_bf16(xT)

        def wslice(W, bias):
            w = np.zeros((KC * 128, HPC * HEAD_DIM), dtype=np.float32)
            w[:HIDDEN] = W[:, cs]
            if has_bias:
                w[HIDDEN] = bias[cs]
            return _to_bf16(w)

        m = {
            "xT": xT,
            "wq": wslice(Wq, bq),
            "wk": wslice(Wk, bk),
            "wv": wslice(Wv, bv),
        }
        if use_mask:
            mask = np.maximum(attention_mask[b, 0, 0, :], -30.0).astype(np.float32)
            mcol = np.ascontiguousarray(mask.reshape(NJ, 128).T)
            m["emb"] = mcol
            m["em2"] = (1.0 + mcol * EXPC).astype(np.float32)
        in_maps.append(m)

    res = run_bass_kernel_spmd(
        nc, in_maps, list(range(8)),
        trace=bool(os.environ.get("KERNEL_TRACE")),
    )
    last_results = res

    out = np.empty((B, S, HIDDEN), dtype=np.float32)
    for c in range(8):
        b = c // 2
        hg = c % 2
        r = res.results[c]["out"]  # [6, 64, 2048]
        out[b, :, hg * HPC * HEAD_DIM:(hg + 1) * HPC * HEAD_DIM] = (
            r.transpose(2, 0, 1).reshape(S, HPC * HEAD_DIM)
        )
    return out


# revision 23
# speedup vs baseline: 1.1157x; 1.1157x over previous
"""BERT self-attention (B=4, S=2048, H=768, 12 heads x d=64) on 8 Trainium2
NeuronCores.

Sharding: core c handles batch b = c//2 and head group hg = c%2 (6 heads).
Each core computes q/k/v projections for its 6 heads from its batch's
hidden_states, then attention per head. No cross-core communication; the host
scatters inputs and gathers/reassembles the output.

Engine balance (the softmax exp on 25.2M elements is the co-bottleneck with
the PE):
  - PE: projections (KC=6 f32r chunks when biases are zero), scores as
    row-tiled K=64 pairs (two heads concurrently in PE row groups 0-63 /
    64-127), ctx with M=128 stationary (64 v-dims | 64 ones columns; the ones
    leave 64 copies of sumexp in psum rows 64-127).
  - ACT: exp for 12 of 16 j-tiles per chunk (exact, with additive mask as
    activation bias when present).
  - DVE: exp for 4 j-tiles via two chained custom DVE ops computing
    (1 + x*scale/16384)^16384 (error ~x^2/32768, well under bf16 noise),
    plus the finalize: reciprocal_approx_fast on the sumexp rows and one
    [64,512] multiply per head.
  - GPSIMD: all psum->sbuf copies (q/k/v projections) and the ones memset.

PE emission is woven: per (pair, i-chunk) period the stream is
[scores jt | ctx(prev) h0 jt | ctx(prev) h1 jt] x 16 with projection matmuls
injected in early periods, so the PE never waits on the elementwise engines.
"""
import os

import numpy as np

if not os.environ.get("KERNEL_TRACE"):
    os.environ.setdefault("BASS_NEVER_TRACE", "1")

import concourse.bass as bass
import concourse.bass_utils as _bass_utils
import concourse.mybir as mybir
import concourse.tile as tile
from concourse import bacc
from concourse.bass import ts
from concourse.bass_utils import run_bass_kernel_spmd

if os.environ.get("KERNEL_LDW_OPT", "0") == "1" and not getattr(
    _bass_utils, "_ant_ldw_patched", False
):
    # enable the walrus LDWEIGHTS double-buffering optimization: without it
    # every full-row-group matmul serializes its 107ns stationary load
    _orig_run_command = _bass_utils.run_command

    def _ldw_run_command(argv, **kwargs):
        argv = [
            "--enable-ldw-opt=true" if a == "--enable-ldw-opt=false" else a
            for a in argv
        ]
        return _orig_run_command(argv, **kwargs)

    _bass_utils.run_command = _ldw_run_command
    _bass_utils._ant_ldw_patched = True

F32R = mybir.dt.float32r
F32 = mybir.dt.float32
BF16 = mybir.dt.bfloat16

HIDDEN = 768
N_HEADS = 12
HEAD_DIM = 64
B = 4
S = 2048
HPC = 6          # heads per core
NI = S // 512    # 4 i-chunks of 512
NJ = S // 128    # 16 j-tiles of 128
SCALE = 1.0 / np.sqrt(HEAD_DIM)
EXPN = 16384.0   # (1 + x*SCALE/EXPN)^EXPN ~= exp(x*SCALE)
EXPC = SCALE / EXPN

DVE_JTS = tuple(
    int(x) for x in os.environ.get("KERNEL_DVE_JTS", "7,12").split(",") if x != ""
)
DVE_JTS_LATE = tuple(
    int(x)
    for x in os.environ.get("KERNEL_DVE_JTS_LATE", "3,7,11,15").split(",")
    if x != ""
)

_cache = {}
last_results = None
_exp_ops = None


def _to_bf16(x):
    import ml_dtypes

    return np.asarray(x, dtype=np.float32).astype(ml_dtypes.bfloat16)


def _register_exp_ops():
    """Register the two chained custom DVE ops for the approximate exp:
      EXP_A: y = (s1 + x*s0)^64      (affine + 6 squarings, 8 v3 stages)
      EXP_B: y = x^256               (8 squarings, 8 v3 stages)
    Composition: (1 + x*EXPC)^16384. Idempotent across imports."""
    global _exp_ops
    if _exp_ops is not None:
        return _exp_ops
    from concourse import dve_ops
    from concourse.dve_spec import Spec, Src0, C0, C1, _has_src1, lower
    from concourse.dve_uop import DveOpSpec

    def build(name, spec):
        existing = [op for op in dve_ops.OPS if op.name == name]
        if existing:
            return existing[0]
        shas = {}
        for ver in ("v3", "v4"):
            try:
                u = lower(spec, ver=ver)
                shas[ver] = DveOpSpec(
                    name=name, opcode=0, uops=u, rd1_en=_has_src1(spec)
                ).sha(ver)
            except Exception:
                pass
        op = dve_ops.DveOp(name, spec, subdim=False, uops_sha=shas)
        dve_ops.OPS.append(op)
        dve_ops.CUSTOM_DVE_SPECS[name] = spec
        dve_ops._SUB_OPCODE_FOR_NAME[name] = (
            dve_ops._CUSTOM_DVE_ROW_BASE + len(dve_ops.OPS) - 1
        )
        return op

    y = C1 + Src0 * C0
    for _ in range(6):
        y = y * y
    spec_a = Spec(
        body=y,
        reference=lambda in0, in1, s0, s1, imm2: (s1 + in0 * s0) ** 64,
    )
    z = Src0
    for _ in range(8):
        z = z * z
    spec_b = Spec(
        body=z,
        reference=lambda in0, in1, s0, s1, imm2: in0 ** 256,
    )
    _exp_ops = (build("ANT_EXP_A", spec_a), build("ANT_EXP_B", spec_b))
    return _exp_ops


def _build(use_mask: bool, has_bias: bool):
    exp_a, exp_b = _register_exp_ops()
    KC = 7 if has_bias else 6  # contraction chunks; row 768 = ones/bias row

    nc = bacc.Bacc("TRN2", target_bir_lowering=False, debug=False, num_devices=8)

    xT_d = nc.dram_tensor("xT", [KC * 128, S], BF16, kind="ExternalInput")
    wq_d = nc.dram_tensor("wq", [KC * 128, HPC * HEAD_DIM], BF16, kind="ExternalInput")
    wk_d = nc.dram_tensor("wk", [KC * 128, HPC * HEAD_DIM], BF16, kind="ExternalInput")
    wv_d = nc.dram_tensor("wv", [KC * 128, HPC * HEAD_DIM], BF16, kind="ExternalInput")
    if use_mask:
        emb_d = nc.dram_tensor("emb", [128, NJ], F32, kind="ExternalInput")
        em2_d = nc.dram_tensor("em2", [128, NJ], F32, kind="ExternalInput")
    out_d = nc.dram_tensor("out", [HPC, HEAD_DIM, S], F32, kind="ExternalOutput")

    with tile.TileContext(nc) as tc:
        with (
            tc.tile_pool(name="const", bufs=1) as cpool,
            tc.tile_pool(name="qk", bufs=1) as qkpool,
            tc.tile_pool(name="vp", bufs=1) as vpool,
            tc.tile_pool(name="op", bufs=3) as opool,
            tc.tile_pool(name="rp", bufs=2) as rpool,
            tc.tile_pool(name="xw", bufs=1) as xwpool,
            tc.tile_pool(name="ex", bufs=34) as expool,
            tc.tile_pool(name="sc", bufs=3) as scpool,
            tc.tile_pool(name="pss", bufs=3, space="PSUM") as pss,
            tc.tile_pool(name="psc", bufs=2, space="PSUM") as psc,
        ):
            if use_mask:
                emb = cpool.tile([128, NJ], F32)
                em2 = cpool.tile([128, NJ], F32)
                nc.sync.dma_start(emb[:], emb_d[:])
                nc.sync.dma_start(em2[:], em2_d[:])

            qT = qkpool.tile([128, HPC // 2, S], BF16)
            kT = qkpool.tile([128, HPC // 2, S], BF16)
            v = vpool.tile([128, NJ, HPC, 128], BF16)
            nc.vector.memset(v[:, :, :, 0:HEAD_DIM], 1.0)

            xT = xwpool.tile([128, KC, S], BF16)
            wq = xwpool.tile([128, KC, HPC * HEAD_DIM], BF16)
            wk = xwpool.tile([128, KC, HPC * HEAD_DIM], BF16)
            wv = xwpool.tile([128, KC, HPC * HEAD_DIM], BF16)
            for c in range(KC):
                nc.sync.dma_start(xT[:, c, :], xT_d[ts(c, 128), :])
                nc.sync.dma_start(wq[:, c, :], wq_d[ts(c, 128), :])
                nc.sync.dma_start(wk[:, c, :], wk_d[ts(c, 128), :])
            for c in range(KC):
                nc.sync.dma_start(wv[:, c, :], wv_d[ts(c, 128), :])

            # warm-up: keep the PE busy during the input DMA window so the
            # HAM clock gate is released before real matmuls start; fp32
            # operands run at 4 cycles/row so 14 matmuls span ~13us; results
            # are discarded (psum slot recycles into the pool)
            wsrc = cpool.tile([128, 640], F32)
            nc.vector.memset(wsrc[:], 0.0)
            wps = pss.tile([128, 2, 512], F32, tag="s", name="warm")
            for wi in range(18):
                nc.tensor.matmul(
                    wps[:, wi % 2, :], wsrc[:, 0:128], wsrc[:, 128:640],
                    start=True, stop=True,
                )

            def emit_qk_mms(p, which):
                # projection matmul thunks, one per (dst, half, c-pair):
                # stationary w chunk [128,128], moving xT [128,512]
                thunks = []
                for w_, dst in [((wq, qT), (wk, kT))[w] for w in which]:
                    for half in range(2):
                        def grp(w_=w_, dst=dst, half=half):
                            acc = pss.tile([128, 2, 512], F32, tag="s")
                            for c in range(KC):
                                for n2 in range(2):
                                    n = 2 * half + n2
                                    nc.tensor.matmul(
                                        acc[:, n2, :], w_[:, c, ts(p, 128)],
                                        xT[:, c, ts(n, 512)],
                                        start=(c == 0), stop=(c == KC - 1),
                                    )
                            nc.vector.tensor_copy(
                                dst[:, p, ts(half, 1024)],
                                acc[:].rearrange("p a n -> p (a n)"),
                            )
                        thunks.append(grp)
                return thunks

            def emit_v_mms(jts):
                thunks = []
                for jt in jts:
                    def grp(jt=jt):
                        pv = pss.tile([128, HPC * HEAD_DIM], F32, tag="s")
                        for c in range(KC):
                            nc.tensor.matmul(
                                pv[:], xT[:, c, ts(jt, 128)], wv[:, c, :],
                                start=(c == 0), stop=(c == KC - 1),
                            )
                        nc.vector.tensor_copy(
                            v[:, jt, :, HEAD_DIM:128],
                            pv[:].rearrange("p (h e) -> p h e", h=HPC),
                        )
                    thunks.append(grp)
                return thunks

            def emit_scores_exp(pr_, ic, jt, dve_set):
                # scores for both heads of the pair as row-tiled K=64 matmuls
                ss = pss.tile([128, 2, 512], F32, tag="s")
                for a_ in range(2):
                    po = 64 * a_
                    nc.tensor.matmul(
                        ss[:, a_, :],
                        kT[po:po + 64, pr_, ts(jt, 128)],
                        qT[po:po + 64, pr_, ts(ic, 512)],
                        start=True, stop=True,
                    )
                ex = expool.tile([128, 2, 512], BF16, tag="e")
                if jt in dve_set:
                    sct = scpool.tile([128, 1024], F32, tag="sc")
                    s1 = em2[:, jt:jt + 1] if use_mask else 1.0
                    nc.vector._custom_dve(
                        exp_a, out=sct[:],
                        in0=ss[:].rearrange("p a n -> p (a n)"),
                        s0=EXPC, s1=s1, imm2=0.0,
                    )
                    nc.vector._custom_dve(
                        exp_b, out=ex[:].rearrange("p a n -> p (a n)"),
                        in0=sct[:], s0=0.0, s1=0.0, imm2=0.0,
                    )
                else:
                    bias = emb[:, jt:jt + 1] if use_mask else 0.0
                    nc.scalar.activation(
                        ex[:], ss[:],
                        mybir.ActivationFunctionType.Exp,
                        scale=SCALE, bias=bias,
                    )
                return ex

            ctx_pc = {}

            def emit_ctx_mm(pr_, a_, jt, ex):
                h = 2 * pr_ + a_
                if jt == 0:
                    ctx_pc[a_] = psc.tile([128, 512], F32, tag="c", name="pc")
                nc.tensor.matmul(
                    ctx_pc[a_][:], v[:, jt, h, :], ex[:, a_, :],
                    start=(jt == 0), stop=(jt == NJ - 1),
                )

            fin_mode = os.environ.get("KERNEL_FINALIZE", "fast")

            def emit_finalize(pr_, ic, heads=(0, 1)):
                for a_ in heads:
                    h = 2 * pr_ + a_
                    pc = ctx_pc[a_]
                    rc = rpool.tile([64, 512], F32, tag="rc")
                    if fin_mode == "fast":
                        nc.vector.reciprocal_approx_fast(rc[:], pc[0:64, :])
                    else:
                        nc.vector.reciprocal(rc[:], pc[0:64, :])
                    o = opool.tile([64, 512], F32, tag="o")
                    nc.vector.tensor_tensor(
                        o[:], pc[64:128, :], rc[:], op=mybir.AluOpType.mult
                    )
                    nc.sync.dma_start(out_d[h, :, ts(ic, 512)], o[:])

            # ---- emission schedule ----
            # pre-loop: full q/k projection for pair 0 (k first: scores need
            # the full kT stationary, q half 0 feeds i-chunks 0-1)
            for t in emit_qk_mms(0, (1, 0)):
                t()

            chunks = [(p_, i_) for p_ in range(HPC // 2) for i_ in range(NI)]
            # projection thunks woven into early periods (v for all j-tiles
            # must land before ctx of chunk 0 runs in period 1)
            fill = {
                0: emit_v_mms(range(0, NJ)),
                1: emit_qk_mms(1, (1,)),
                2: emit_qk_mms(1, (0,)),
                3: emit_qk_mms(2, (1,)),
                4: emit_qk_mms(2, (0,)),
            }
            prev = None
            for ci, ch in enumerate(chunks):
                pr_, ic = ch
                fill_t = list(fill.get(ci, []))
                # weave: scores(chunk ci) jt | ctx(chunk ci-1) h0/h1 jt | fill
                nfill = len(fill_t)
                if ci == 0:
                    dve_set = ()
                elif ci < 5:
                    dve_set = DVE_JTS
                else:
                    dve_set = DVE_JTS_LATE
                exs = []
                for jt in range(NJ):
                    exs.append(emit_scores_exp(pr_, ic, jt, dve_set))
                    if prev is not None:
                        # head 0's ctx runs in the first half of the period and
                        # finalizes mid-period, freeing its psum bank well
                        # before the next period's ctx needs it; head 1 second
                        a_, j0 = (0, 2 * jt) if jt < 8 else (1, 2 * (jt - 8))
                        emit_ctx_mm(prev[0][0], a_, j0, prev[1][j0])
                        emit_ctx_mm(prev[0][0], a_, j0 + 1, prev[1][j0 + 1])
                        if jt == 7:
                            emit_finalize(prev[0][0], prev[0][1], heads=(0,))
                        elif jt == 15:
                            emit_finalize(prev[0][0], prev[0][1], heads=(1,))
                    # inject projection work spread across the period
                    target = nfill * (jt + 1) // NJ
                    while nfill - len(fill_t) < target:
                        fill_t.pop(0)()
                while fill_t:
                    fill_t.pop(0)()
                prev = (ch, exs)
            # drain: ctx + finalize for the last chunk
            for a_ in range(2):
                for jt in range(NJ):
                    emit_ctx_mm(prev[0][0], a_, jt, prev[1][jt])
                emit_finalize(prev[0][0], prev[0][1], heads=(a_,))

    nc.compile()
    return nc


def _get_nc(use_mask: bool, has_bias: bool):
    key = (use_mask, has_bias)
    if key not in _cache:
        _cache[key] = _build(use_mask, has_bias)
    return _cache[key]


def kernel(hidden_states, attention_mask, Wq, bq, Wk, bk, Wv, bv):
    global last_results
    hidden_states = np.asarray(hidden_states, dtype=np.float32)
    attention_mask = np.asarray(attention_mask, dtype=np.float32)
    Wq = np.asarray(Wq, dtype=np.float32)
    Wk = np.asarray(Wk, dtype=np.float32)
    Wv = np.asarray(Wv, dtype=np.float32)
    bq = np.asarray(bq, dtype=np.float32)
    bk = np.asarray(bk, dtype=np.float32)
    bv = np.asarray(bv, dtype=np.float32)

    use_mask = bool(np.any(attention_mask))
    has_bias = bool(np.any(bq) or np.any(bk) or np.any(bv))
    nc = _get_nc(use_mask, has_bias)
    KC = 7 if has_bias else 6

    in_maps = []
    for c in range(8):
        b = c // 2
        hg = c % 2
        cs = slice(hg * HPC * HEAD_DIM, (hg + 1) * HPC * HEAD_DIM)

        xT = np.zeros((KC * 128, S), dtype=np.float32)
        xT[:HIDDEN] = hidden_states[b].T
        if has_bias:
            xT[HIDDEN] = 1.0
        xT = _to_bf16(xT)

        def wslice(W, bias):
            w = np.zeros((KC * 128, HPC * HEAD_DIM), dtype=np.float32)
            w[:HIDDEN] = W[:, cs]
            if has_bias:
                w[HIDDEN] = bias[cs]
            return _to_bf16(w)

        m = {
            "xT": xT,
            "wq": wslice(Wq, bq),
            "wk": wslice(Wk, bk),
            "wv": wslice(Wv, bv),
        }
        if use_mask:
            mask = np.maximum(attention_mask[b, 0, 0, :], -30.0).astype(np.float32)
            mcol = np.ascontiguousarray(mask.reshape(NJ, 128).T)
            m["emb"] = mcol
            m["em2"] = (1.0 + mcol * EXPC).astype(np.float32)
        in_maps.append(m)

    res = run_bass_kernel_spmd(
        nc, in_maps, list(range(8)),
        trace=bool(os.environ.get("KERNEL_TRACE")),
    )
    last_results = res

    out = np.empty((B, S, HIDDEN), dtype=np.float32)
    for c in range(8):
        b = c // 2
        hg = c % 2
        r = res.results[c]["out"]  # [6, 64, 2048]
        out[b, :, hg * HPC * HEAD_DIM:(hg + 1) * HPC * HEAD_DIM] = (
            r.transpose(2, 0, 1).reshape(S, HPC * HEAD_DIM)
        )
    return out


# revision 24
# speedup vs baseline: 1.1344x; 1.0168x over previous
"""BERT self-attention (B=4, S=2048, H=768, 12 heads x d=64) on 8 Trainium2
NeuronCores.

Sharding: core c handles batch b = c//2 and head group hg = c%2 (6 heads).
Each core computes q/k/v projections for its 6 heads from its batch's
hidden_states, then attention per head. No cross-core communication; the host
scatters inputs and gathers/reassembles the output.

Engine balance (the softmax exp on 25.2M elements is the co-bottleneck with
the PE):
  - PE: projections (KC=6 f32r chunks when biases are zero), scores as
    row-tiled K=64 pairs (two heads concurrently in PE row groups 0-63 /
    64-127), ctx with M=128 stationary (64 v-dims | 64 ones columns; the ones
    leave 64 copies of sumexp in psum rows 64-127).
  - ACT: exp for 12 of 16 j-tiles per chunk (exact, with additive mask as
    activation bias when present).
  - DVE: exp for 4 j-tiles via two chained custom DVE ops computing
    (1 + x*scale/16384)^16384 (error ~x^2/32768, well under bf16 noise),
    plus the finalize: reciprocal_approx_fast on the sumexp rows and one
    [64,512] multiply per head.
  - GPSIMD: all psum->sbuf copies (q/k/v projections) and the ones memset.

PE emission is woven: per (pair, i-chunk) period the stream is
[scores jt | ctx(prev) h0 jt | ctx(prev) h1 jt] x 16 with projection matmuls
injected in early periods, so the PE never waits on the elementwise engines.
"""
import os

import numpy as np

if not os.environ.get("KERNEL_TRACE"):
    os.environ.setdefault("BASS_NEVER_TRACE", "1")

import concourse.bass as bass
import concourse.bass_utils as _bass_utils
import concourse.mybir as mybir
import concourse.tile as tile
from concourse import bacc
from concourse.bass import ts
from concourse.bass_utils import run_bass_kernel_spmd

if os.environ.get("KERNEL_LDW_OPT", "0") == "1" and not getattr(
    _bass_utils, "_ant_ldw_patched", False
):
    # enable the walrus LDWEIGHTS double-buffering optimization: without it
    # every full-row-group matmul serializes its 107ns stationary load
    _orig_run_command = _bass_utils.run_command

    def _ldw_run_command(argv, **kwargs):
        argv = [
            "--enable-ldw-opt=true" if a == "--enable-ldw-opt=false" else a
            for a in argv
        ]
        return _orig_run_command(argv, **kwargs)

    _bass_utils.run_command = _ldw_run_command
    _bass_utils._ant_ldw_patched = True

F32R = mybir.dt.float32r
F32 = mybir.dt.float32
BF16 = mybir.dt.bfloat16

HIDDEN = 768
N_HEADS = 12
HEAD_DIM = 64
B = 4
S = 2048
HPC = 6          # heads per core
NI = S // 512    # 4 i-chunks of 512
NJ = S // 128    # 16 j-tiles of 128
SCALE = 1.0 / np.sqrt(HEAD_DIM)
EXPN = 16384.0   # (1 + x*SCALE/EXPN)^EXPN ~= exp(x*SCALE)
EXPC = SCALE / EXPN

DVE_JTS = tuple(
    int(x) for x in os.environ.get("KERNEL_DVE_JTS", "7,12").split(",") if x != ""
)
DVE_JTS_LATE = tuple(
    int(x)
    for x in os.environ.get("KERNEL_DVE_JTS_LATE", "3,7,11,15").split(",")
    if x != ""
)

_cache = {}
last_results = None
_exp_ops = None


def _to_bf16(x):
    import ml_dtypes

    return np.asarray(x, dtype=np.float32).astype(ml_dtypes.bfloat16)


def _register_exp_ops():
    """Register the two chained custom DVE ops for the approximate exp:
      EXP_A: y = (s1 + x*s0)^64      (affine + 6 squarings, 8 v3 stages)
      EXP_B: y = x^256               (8 squarings, 8 v3 stages)
    Composition: (1 + x*EXPC)^16384. Idempotent across imports."""
    global _exp_ops
    if _exp_ops is not None:
        return _exp_ops
    from concourse import dve_ops
    from concourse.dve_spec import Spec, Src0, C0, C1, _has_src1, lower
    from concourse.dve_uop import DveOpSpec

    def build(name, spec):
        existing = [op for op in dve_ops.OPS if op.name == name]
        if existing:
            return existing[0]
        shas = {}
        for ver in ("v3", "v4"):
            try:
                u = lower(spec, ver=ver)
                shas[ver] = DveOpSpec(
                    name=name, opcode=0, uops=u, rd1_en=_has_src1(spec)
                ).sha(ver)
            except Exception:
                pass
        op = dve_ops.DveOp(name, spec, subdim=False, uops_sha=shas)
        dve_ops.OPS.append(op)
        dve_ops.CUSTOM_DVE_SPECS[name] = spec
        dve_ops._SUB_OPCODE_FOR_NAME[name] = (
            dve_ops._CUSTOM_DVE_ROW_BASE + len(dve_ops.OPS) - 1
        )
        return op

    y = C1 + Src0 * C0
    for _ in range(6):
        y = y * y
    spec_a = Spec(
        body=y,
        reference=lambda in0, in1, s0, s1, imm2: (s1 + in0 * s0) ** 64,
    )
    z = Src0
    for _ in range(8):
        z = z * z
    spec_b = Spec(
        body=z,
        reference=lambda in0, in1, s0, s1, imm2: in0 ** 256,
    )
    _exp_ops = (build("ANT_EXP_A", spec_a), build("ANT_EXP_B", spec_b))
    return _exp_ops


def _build(use_mask: bool, has_bias: bool):
    exp_a, exp_b = _register_exp_ops()
    KC = 7 if has_bias else 6  # contraction chunks; row 768 = ones/bias row

    nc = bacc.Bacc("TRN2", target_bir_lowering=False, debug=False, num_devices=8)

    xT_d = nc.dram_tensor("xT", [KC * 128, S], BF16, kind="ExternalInput")
    wq_d = nc.dram_tensor("wq", [KC * 128, HPC * HEAD_DIM], BF16, kind="ExternalInput")
    wk_d = nc.dram_tensor("wk", [KC * 128, HPC * HEAD_DIM], BF16, kind="ExternalInput")
    wv_d = nc.dram_tensor("wv", [KC * 128, HPC * HEAD_DIM], BF16, kind="ExternalInput")
    if use_mask:
        emb_d = nc.dram_tensor("emb", [128, NJ], F32, kind="ExternalInput")
        em2_d = nc.dram_tensor("em2", [128, NJ], F32, kind="ExternalInput")
    out_d = nc.dram_tensor("out", [HPC, HEAD_DIM, S], F32, kind="ExternalOutput")

    with tile.TileContext(nc) as tc:
        with (
            tc.tile_pool(name="const", bufs=1) as cpool,
            tc.tile_pool(name="qk", bufs=1) as qkpool,
            tc.tile_pool(name="vp", bufs=1) as vpool,
            tc.tile_pool(name="op", bufs=3) as opool,
            tc.tile_pool(name="rp", bufs=2) as rpool,
            tc.tile_pool(name="xw", bufs=1) as xwpool,
            tc.tile_pool(name="ex", bufs=34) as expool,
            tc.tile_pool(name="sc", bufs=3) as scpool,
            tc.tile_pool(name="pss", bufs=3, space="PSUM") as pss,
            tc.tile_pool(name="psc", bufs=2, space="PSUM") as psc,
        ):
            if use_mask:
                emb = cpool.tile([128, NJ], F32)
                em2 = cpool.tile([128, NJ], F32)
                nc.sync.dma_start(emb[:], emb_d[:])
                nc.sync.dma_start(em2[:], em2_d[:])

            qT = qkpool.tile([128, HPC // 2, S], BF16)
            kT = qkpool.tile([128, HPC // 2, S], BF16)
            v = vpool.tile([128, NJ, HPC, 128], BF16)
            nc.vector.memset(v[:, :, :, 0:HEAD_DIM], 1.0)

            xT = xwpool.tile([128, KC, S], BF16)
            wq = xwpool.tile([128, KC, HPC * HEAD_DIM], BF16)
            wk = xwpool.tile([128, KC, HPC * HEAD_DIM], BF16)
            wv = xwpool.tile([128, KC, HPC * HEAD_DIM], BF16)
            for c in range(KC):
                nc.sync.dma_start(xT[:, c, :], xT_d[ts(c, 128), :])
                nc.sync.dma_start(wq[:, c, :], wq_d[ts(c, 128), :])
                nc.sync.dma_start(wk[:, c, :], wk_d[ts(c, 128), :])
            for c in range(KC):
                nc.sync.dma_start(wv[:, c, :], wv_d[ts(c, 128), :])

            # warm-up: keep the PE busy during the input DMA window so the
            # HAM clock gate is released before real matmuls start; fp32
            # operands run at 4 cycles/row so 14 matmuls span ~13us; results
            # are discarded (psum slot recycles into the pool)
            wsrc = cpool.tile([128, 640], F32)
            nc.vector.memset(wsrc[:], 0.0)
            wps = pss.tile([128, 2, 512], F32, tag="s", name="warm")
            for wi in range(14):
                nc.tensor.matmul(
                    wps[:, wi % 2, :], wsrc[:, 0:128], wsrc[:, 128:640],
                    start=True, stop=True,
                )

            def emit_qk_mms(p, which):
                # projection matmul thunks, one per (dst, half, c-pair):
                # stationary w chunk [128,128], moving xT [128,512]
                thunks = []
                for w_, dst in [((wq, qT), (wk, kT))[w] for w in which]:
                    for half in range(2):
                        def grp(w_=w_, dst=dst, half=half):
                            acc = pss.tile([128, 2, 512], F32, tag="s")
                            for c in range(KC):
                                for n2 in range(2):
                                    n = 2 * half + n2
                                    nc.tensor.matmul(
                                        acc[:, n2, :], w_[:, c, ts(p, 128)],
                                        xT[:, c, ts(n, 512)],
                                        start=(c == 0), stop=(c == KC - 1),
                                    )
                            nc.vector.tensor_copy(
                                dst[:, p, ts(half, 1024)],
                                acc[:].rearrange("p a n -> p (a n)"),
                            )
                        thunks.append(grp)
                return thunks

            def emit_v_mms(jts):
                thunks = []
                for jt in jts:
                    def grp(jt=jt):
                        pv = pss.tile([128, HPC * HEAD_DIM], F32, tag="s")
                        for c in range(KC):
                            nc.tensor.matmul(
                                pv[:], xT[:, c, ts(jt, 128)], wv[:, c, :],
                                start=(c == 0), stop=(c == KC - 1),
                            )
                        nc.vector.tensor_copy(
                            v[:, jt, :, HEAD_DIM:128],
                            pv[:].rearrange("p (h e) -> p h e", h=HPC),
                        )
                    thunks.append(grp)
                return thunks

            def emit_scores_exp(pr_, ic, jt, dve_set):
                # scores for both heads of the pair as row-tiled K=64 matmuls
                ss = pss.tile([128, 2, 512], F32, tag="s")
                for a_ in range(2):
                    po = 64 * a_
                    nc.tensor.matmul(
                        ss[:, a_, :],
                        kT[po:po + 64, pr_, ts(jt, 128)],
                        qT[po:po + 64, pr_, ts(ic, 512)],
                        start=True, stop=True,
                    )
                ex = expool.tile([128, 2, 512], BF16, tag="e")
                if jt in dve_set:
                    sct = scpool.tile([128, 1024], F32, tag="sc")
                    s1 = em2[:, jt:jt + 1] if use_mask else 1.0
                    nc.vector._custom_dve(
                        exp_a, out=sct[:],
                        in0=ss[:].rearrange("p a n -> p (a n)"),
                        s0=EXPC, s1=s1, imm2=0.0,
                    )
                    nc.vector._custom_dve(
                        exp_b, out=ex[:].rearrange("p a n -> p (a n)"),
                        in0=sct[:], s0=0.0, s1=0.0, imm2=0.0,
                    )
                else:
                    bias = emb[:, jt:jt + 1] if use_mask else 0.0
                    nc.scalar.activation(
                        ex[:], ss[:],
                        mybir.ActivationFunctionType.Exp,
                        scale=SCALE, bias=bias,
                    )
                return ex

            ctx_pc = {}

            def emit_ctx_mm(pr_, a_, jt, ex):
                h = 2 * pr_ + a_
                if jt == 0:
                    ctx_pc[a_] = psc.tile([128, 512], F32, tag="c", name="pc")
                nc.tensor.matmul(
                    ctx_pc[a_][:], v[:, jt, h, :], ex[:, a_, :],
                    start=(jt == 0), stop=(jt == NJ - 1),
                )

            fin_mode = os.environ.get("KERNEL_FINALIZE", "fast")

            def emit_finalize(pr_, ic, heads=(0, 1)):
                for a_ in heads:
                    h = 2 * pr_ + a_
                    pc = ctx_pc[a_]
                    rc = rpool.tile([64, 512], F32, tag="rc")
                    if fin_mode == "fast":
                        nc.vector.reciprocal_approx_fast(rc[:], pc[0:64, :])
                    else:
                        nc.vector.reciprocal(rc[:], pc[0:64, :])
                    o = opool.tile([64, 512], F32, tag="o")
                    nc.vector.tensor_tensor(
                        o[:], pc[64:128, :], rc[:], op=mybir.AluOpType.mult
                    )
                    nc.sync.dma_start(out_d[h, :, ts(ic, 512)], o[:])

            # ---- emission schedule ----
            # pre-loop: full q/k projection for pair 0 (k first: scores need
            # the full kT stationary, q half 0 feeds i-chunks 0-1)
            for t in emit_qk_mms(0, (1, 0)):
                t()

            chunks = [(p_, i_) for p_ in range(HPC // 2) for i_ in range(NI)]
            # projection thunks woven into early periods (v for all j-tiles
            # must land before ctx of chunk 0 runs in period 1)
            fill = {
                0: emit_v_mms(range(0, NJ)),
                1: emit_qk_mms(1, (1,)),
                2: emit_qk_mms(1, (0,)),
                3: emit_qk_mms(2, (1,)),
                4: emit_qk_mms(2, (0,)),
            }
            prev = None
            for ci, ch in enumerate(chunks):
                pr_, ic = ch
                fill_t = list(fill.get(ci, []))
                # weave: scores(chunk ci) jt | ctx(chunk ci-1) h0/h1 jt | fill
                nfill = len(fill_t)
                if ci == 0:
                    dve_set = ()
                elif ci < 5:
                    dve_set = DVE_JTS
                else:
                    dve_set = DVE_JTS_LATE
                exs = []
                for jt in range(NJ):
                    exs.append(emit_scores_exp(pr_, ic, jt, dve_set))
                    if prev is not None:
                        # head 0's ctx runs in the first half of the period and
                        # finalizes mid-period, freeing its psum bank well
                        # before the next period's ctx needs it; head 1 second
                        a_, j0 = (0, 2 * jt) if jt < 8 else (1, 2 * (jt - 8))
                        emit_ctx_mm(prev[0][0], a_, j0, prev[1][j0])
                        emit_ctx_mm(prev[0][0], a_, j0 + 1, prev[1][j0 + 1])
                        if jt == 7:
                            emit_finalize(prev[0][0], prev[0][1], heads=(0,))
                        elif jt == 15:
                            emit_finalize(prev[0][0], prev[0][1], heads=(1,))
                    # inject projection work spread across the period
                    target = nfill * (jt + 1) // NJ
                    while nfill - len(fill_t) < target:
                        fill_t.pop(0)()
                while fill_t:
                    fill_t.pop(0)()
                prev = (ch, exs)
            # drain: ctx + finalize for the last chunk
            for a_ in range(2):
                for jt in range(NJ):
                    emit_ctx_mm(prev[0][0], a_, jt, prev[1][jt])
                emit_finalize(prev[0][0], prev[0][1], heads=(a_,))

    nc.compile()
    return nc


def _get_nc(use_mask: bool, has_bias: bool):
    key = (use_mask, has_bias)
    if key not in _cache:
        _cache[key] = _build(use_mask, has_bias)
    return _cache[key]


def kernel(hidden_states, attention_mask, Wq, bq, Wk, bk, Wv, bv):
    global last_results
    hidden_states = np.asarray(hidden_states, dtype=np.float32)
    attention_mask = np.asarray(attention_mask, dtype=np.float32)
    Wq = np.asarray(Wq, dtype=np.float32)
    Wk = np.asarray(Wk, dtype=np.float32)
    Wv = np.asarray(Wv, dtype=np.float32)
    bq = np.asarray(bq, dtype=np.float32)
    bk = np.asarray(bk, dtype=np.float32)
    bv = np.asarray(bv, dtype=np.float32)

    use_mask = bool(np.any(attention_mask))
    has_bias = bool(np.any(bq) or np.any(bk) or np.any(bv))
    nc = _get_nc(use_mask, has_bias)
    KC = 7 if has_bias else 6

    in_maps = []
    for c in range(8):
        b = c // 2
        hg = c % 2
        cs = slice(hg * HPC * HEAD_DIM, (hg + 1) * HPC * HEAD_DIM)

        xT = np.zeros((KC * 128, S), dtype=np.float32)
        xT[:HIDDEN] = hidden_states[b].T
        if has_bias:
            xT[HIDDEN] = 1.0
        xT = _to_bf16(xT)

        def wslice(W, bias):
            w = np.zeros((KC * 128, HPC * HEAD_DIM), dtype=np.float32)
            w[:HIDDEN] = W[:, cs]
            if has_bias:
                w[HIDDEN] = bias[cs]
            return _to_bf16(w)

        m = {
            "xT": xT,
            "wq": wslice(Wq, bq),
            "wk": wslice(Wk, bk),
            "wv": wslice(Wv, bv),
        }
        if use_mask:
            mask = np.maximum(attention_mask[b, 0, 0, :], -30.0).astype(np.float32)
            mcol = np.ascontiguousarray(mask.reshape(NJ, 128).T)
            m["emb"] = mcol
            m["em2"] = (1.0 + mcol * EXPC).astype(np.float32)
        in_maps.append(m)

    res = run_bass_kernel_spmd(
        nc, in_maps, list(range(8)),
        trace=bool(os.environ.get("KERNEL_TRACE")),
    )
    last_results = res

    out = np.empty((B, S, HIDDEN), dtype=np.float32)
    for c in range(8):
        b = c // 2
        hg = c % 2
        r = res.results[c]["out"]  # [6, 64, 2048]
        out[b, :, hg * HPC * HEAD_DIM:(hg + 1) * HPC * HEAD_DIM] = (
            r.transpose(2, 0, 1).reshape(S, HPC * HEAD_DIM)
        )
    return out
